# revision 31
# baseline (speedup 1.0000x reference)
"""Single-head causal attention (B=8, T=2048, C=1024, H=128) on 8 TRN2 NeuronCores.

Sharding: data-parallel over batch B — core b computes attention for x[b].
Host-side prep per core: x[b] is transposed to xT [C, T] (contraction dim C on
SBUF partitions) and the softmax scale C**-0.5 is folded into Wq. The kernel
returns the UNNORMALIZED attention output pavT [H, T] (bf16) plus the softmax
denominators sums [1, T] (f32); the host divides and untransposes.

Device kernel per core (v2 — pair-tile attention, warmup MMs, staged DMA):
  head: input DMAs staged across the 3 queues (sync/scalar/gpsimd) in
    consumption order; ~16 warmup matmuls on a dummy tile keep the PE HAM
    clock warm through the DMA-bound head so the real stream runs at 2.4 GHz.
  quarter 0 projections c-major over arriving x chunks (V,Q,K per chunk).
  attention per q-block g over PAIRS of s-tiles (j, j+1):
      ST pair = two matmuls into one 2-bank PSUM tile [128, 1024]
      diag boundary subtiles get triangular mask adds (DVE, in PSUM)
      ONE exp ACTIVATE over the pair -> SBUF bf16 (halves ACT overhead);
      masked strips zeroed by DVE memsets
      pav += V_j.T @ exp per tile [H, 512] PSUM accumulate (PE)
      running exp-sums: pair-width tensor_adds, DVE (even pairs) / GpSimd
      interleaved projection matmuls fill the exp latency
  blocks 0-2 finalize deferred into the next block (fold + ones-col matmul +
  copies + DMA). block 3 finalizes sums EARLY via accumulating ones-col
  matmuls into PSUM, so the kernel tail is just 2 matmuls + copy + DMA.
"""

from contextlib import ExitStack

import numpy as np
import ml_dtypes

B, T, C, H = 8, 2048, 1024, 128
P = 128
NT = T // P  # 16 s-tiles
NCC = C // P  # 8 contraction chunks
QB = 512  # q-block width
NQB = T // QB  # 4 q-blocks / projection quarters
N_CORES = 8
SCALE = float(C) ** -0.5

_CACHE = {}


def _build():
    import concourse.bass as bass
    import concourse.mybir as mybir
    import concourse.tile as tile
    from concourse import bacc

    dt = mybir.dt
    dt_in = dt.bfloat16
    dt_av = dt.bfloat16
    f32 = dt.float32

    nc = bacc.Bacc("TRN2", target_bir_lowering=False, debug=False)
    xT = nc.dram_tensor("xT", [C, T], dt_in, kind="ExternalInput").ap()
    wq = nc.dram_tensor("wq", [P, NCC * H], dt_in, kind="ExternalInput").ap()
    wk = nc.dram_tensor("wk", [P, NCC * H], dt_in, kind="ExternalInput").ap()
    wv = nc.dram_tensor("wv", [P, NCC * H], dt_in, kind="ExternalInput").ap()
    pavT = nc.dram_tensor("pavT", [H, T], dt_av, kind="ExternalOutput").ap()
    sums = nc.dram_tensor("sums", [1, T], f32, kind="ExternalOutput").ap()

    with tile.TileContext(nc) as tc, ExitStack() as ctx:
        wpool = ctx.enter_context(tc.tile_pool(name="wpool", bufs=1))
        w_sb = {
            name: wpool.tile([P, NCC * H], dt_in, name=f"{name}_sb")
            for name in ("wv", "wq", "wk")
        }
        xpool = ctx.enter_context(tc.tile_pool(name="xpool", bufs=1))
        xt_sb = xpool.tile([P, NCC * T], dt_in)
        xt_v = xt_sb.rearrange("p (c t) -> p c t", c=NCC)
        xT_v = xT.rearrange("(c p) t -> p c t", c=NCC)

        def x_chunk(eng, c0, c1, t0, t1):
            eng.dma_start(xt_v[:, c0:c1, t0:t1], xT_v[:, c0:c1, t0:t1])

        # --- input DMAs: quarter-0 pieces spread in consumption order over
        # the 3 queues, with per-queue byte counts balanced (~615KB each) so
        # all queues finish the quarter-0 phase together and the bulk n=1..3
        # loads (queued behind) don't steal bandwidth from it ---
        x_chunk(nc.sync, 0, 1, 0, 512)  # c0 first: gates the first proj MM
        nc.scalar.dma_start(w_sb["wv"], wv)
        x_chunk(nc.gpsimd, 1, 2, 0, 512)
        x_chunk(nc.scalar, 2, 3, 0, 512)
        x_chunk(nc.sync, 3, 4, 0, 512)
        nc.gpsimd.dma_start(w_sb["wq"], wq)
        x_chunk(nc.scalar, 5, 6, 0, 512)
        x_chunk(nc.sync, 6, 7, 0, 512)
        x_chunk(nc.gpsimd, 4, 5, 0, 512)
        nc.sync.dma_start(w_sb["wk"], wk)
        x_chunk(nc.scalar, 7, 8, 0, 512)
        # n=1..3: behind the quarter-0 items in each queue's FIFO
        x_chunk(nc.gpsimd, 0, 4, 512, 1024)
        x_chunk(nc.sync, 4, 8, 512, 1024)
        x_chunk(nc.gpsimd, 0, 8, 1024, 1536)
        x_chunk(nc.sync, 0, 8, 1536, 2048)

        consts = ctx.enter_context(tc.tile_pool(name="consts", bufs=1))
        # triangular boundary mask: tri[s, q] = -30000 where q < s else 0
        tri = consts.tile([P, P], f32)
        nc.gpsimd.memset(tri, 0.0)
        nc.gpsimd.affine_select(
            out=tri,
            in_=tri,
            compare_op=mybir.AluOpType.is_ge,
            fill=-30000.0,
            base=0,
            pattern=[[1, P]],
            channel_multiplier=-1,
        )
        ones_col = consts.tile([P, 1], dt_av)
        nc.vector.memset(ones_col, 1.0)
        warm = consts.tile([P, QB], dt_in)
        nc.vector.memset(warm, 0.002)

        qkv = ctx.enter_context(tc.tile_pool(name="qkv", bufs=1))
        qt_sb = qkv.tile([P, T], dt_in)
        kt_sb = qkv.tile([P, T], dt_in)
        vt_sb = qkv.tile([P, T], dt_av)
        vpool = ctx.enter_context(tc.tile_pool(name="vpool", bufs=1))
        v_sb = vpool.tile([P, NT * H], dt_av)

        # PSUM budget (8 banks): score pairs 2x2 (also hosts q0/k0 proj),
        # proj 2x1 (also hosts warmup + sums tiles), pav 2x1
        ps_sc = ctx.enter_context(tc.tile_pool(name="ps_sc", bufs=2, space="PSUM"))
        ps_proj = ctx.enter_context(tc.tile_pool(name="ps_proj", bufs=2, space="PSUM"))
        ps_av = ctx.enter_context(tc.tile_pool(name="ps_av", bufs=2, space="PSUM"))

        expp = ctx.enter_context(tc.tile_pool(name="expp", bufs=10))
        outp = ctx.enter_context(tc.tile_pool(name="outp", bufs=2))
        accp = ctx.enter_context(tc.tile_pool(name="accp", bufs=4))
        sums_sb_pool = ctx.enter_context(tc.tile_pool(name="sums_sb", bufs=1))
        sums_sb = sums_sb_pool.tile([1, T], f32)

        # warmup: ONE accumulation group into one proj-pool bank, so the MMs
        # run back-to-back (no WAW semaphores) and HAM reaches 8/8 early.
        # All upfront: ~10 x 427ns cold > the 3.4us HAM SHORT window.
        N_WARM = 24
        warm_ps = ps_proj.tile([P, QB], f32, name="warm_ps", tag="proj")
        for i in range(N_WARM):
            nc.tensor.matmul(
                warm_ps, warm[:, 0:P], warm,
                start=(i == 0), stop=(i == N_WARM - 1),
                skip_group_check=True,
            )

        def proj_ops(pname, dst, n, copy_eng, do_transpose, pool=None, ptag=None):
            """Closures: 8 proj matmuls + copy (+ quarter transpose)."""
            wt = w_sb[pname]
            state = {}
            pool = pool if pool is not None else ps_proj
            ptag = ptag if ptag is not None else "proj"

            def mk_mm(c):
                def op():
                    if c == 0:
                        state["ps"] = pool.tile(
                            [P, QB], f32, name=f"ps_{pname}{n}", tag=ptag
                        )
                    nc.tensor.matmul(
                        state["ps"],
                        wt[:, c * H : (c + 1) * H],
                        xt_sb[:, c * T + n * QB : c * T + (n + 1) * QB],
                        start=(c == 0),
                        stop=(c == NCC - 1),
                    )

                return op

            def cp():
                fn = (
                    copy_eng.tensor_copy if copy_eng is nc.vector else copy_eng.copy
                )
                fn(dst[:, n * QB : (n + 1) * QB], state["ps"])

            ops = [mk_mm(c) for c in range(NCC)] + [cp]
            if do_transpose:

                def tr():
                    nc.sync.dma_start(
                        v_sb[:, 4 * n * H : 4 * (n + 1) * H].rearrange(
                            "p (t h) -> p t h", t=4
                        ),
                        vt_sb[:, n * QB : (n + 1) * QB],
                        transpose=True,
                    )

                ops.append(tr)
            return ops

        # --- quarter 0 up front, GROUP-major (V, then Q, then K): the V
        # path has the longest tail (copy + DMA-transpose + receipt), so V's
        # group must finish first; each group's copy is emitted immediately.
        # Q0/K0 accumulate in the (idle until block 0) score-pair banks so
        # the three groups don't fight over the two proj banks.
        v0 = proj_ops("wv", vt_sb, 0, nc.scalar, True)
        q0 = proj_ops("wq", qt_sb, 0, nc.vector, False, pool=ps_sc, ptag="sc")
        k0 = proj_ops("wk", kt_sb, 0, nc.vector, False, pool=ps_sc, ptag="sc")
        for op in v0 + q0 + k0:
            op()

        # --- attention blocks (pair-tile) ---
        deferred = []  # sums finalization + outputs of the previous block
        for g in range(NQB):
            qs0 = g * QB
            njt = 4 * g + 4
            npr = njt // 2
            pav = ps_av.tile([P, QB], f32, name=f"pav{g}", tag="ps_av")

            # pair processing order: clean pairs ascending, diag pairs last;
            # last block puts the diag pairs mid-block so the kernel ends on
            # dense full tiles
            pairs = [(2 * i, 2 * i + 1) for i in range(npr)]
            if g == NQB - 1:
                # natural order: the kernel ends on the small diagonal pair
                # (14,15), so the tail exp is short and the final sums
                # matmuls are tiny.  K(3)/V(3) must land by pair idx 6.
                win = 3
            else:
                win = max(npr - 1, 1)

            # deadline-ordered proj op queue for this block.  V(3) is
            # projected in block 3 itself (not block 2): block 2 is
            # PE-bound and block 3 ACT-bound, so this balances them.
            # Its copy goes on DVE to keep ACT free for block-3 exps.
            ops_q = []
            if g == NQB - 1:
                # V(3) first: its copy+transpose+receipt chain is longest
                ops_q += proj_ops("wv", vt_sb, g, nc.vector, True)
            if g >= 1:
                ops_q += proj_ops("wk", kt_sb, g, nc.vector, False)
            if g + 1 < NQB:
                ops_q += proj_ops("wq", qt_sb, g + 1, nc.vector, False)
                if g + 1 < NQB - 1:
                    ops_q += proj_ops("wv", vt_sb, g + 1, nc.scalar, True)

            # acc lanes: even pairs -> DVE, odd pairs -> GpSimd.  For the
            # last block, pair npr-1 is left out of the lanes (it feeds the
            # ones-matmul tail directly).
            lanes = {
                "e": {"eng": nc.vector, "first": None, "acc": None},
                "o": {"eng": nc.gpsimd, "first": None, "acc": None},
            }
            # GpSimd ("o") takes EARLY pairs only — its adds are ~3x slower
            # than DVE, so each block's final adds (which feed the sums
            # matmuls) always land on DVE ("e").
            lane_seq = []
            for pi in range(npr):
                if g == NQB - 1 and pi == npr - 1:
                    lane_seq.append(None)
                elif g == NQB - 1:
                    lane_seq.append("o" if pi in (0, 2) else "e")
                else:
                    lane_seq.append("o" if pi % 2 == 0 else "e")

            last_exp_tile = [None]
            pss3_state = [None]

            def acc_tile(lane_key, expt):
                ln = lanes[lane_key]
                eng = ln["eng"]
                if ln["acc"] is None and ln["first"] is None:
                    ln["first"] = (expt,)
                elif ln["acc"] is None:
                    acc = accp.tile(
                        [P, 2 * QB], dt_av, name=f"acc{g}{lane_key}", tag="acc"
                    )
                    # init: acc = first + this (full pair width; strips zeroed)
                    eng.tensor_add(acc, ln["first"][0], expt)
                    ln["acc"] = acc
                    ln["first"] = None
                else:
                    eng.tensor_add(ln["acc"], ln["acc"], expt)

            def drain(k):
                if ops_q and k < win:
                    take = -(-len(ops_q) // (win - k))
                    for op in ops_q[:take]:
                        op()
                    del ops_q[:take]

            for pi, (ja, jb) in enumerate(pairs):
                da, db = ja - 4 * g, jb - 4 * g
                qloA = max(0, P * da)
                qloB = max(0, P * db)
                ps = ps_sc.tile([P, 2 * QB], f32, name=f"ps_{g}_{pi}", tag="sc")
                nc.tensor.matmul(
                    ps[:, qloA:QB],
                    kt_sb[:, ja * P : (ja + 1) * P],
                    qt_sb[:, qs0 + qloA : qs0 + QB],
                    start=True,
                    stop=True,
                )
                if da >= 0:
                    nc.vector.tensor_add(
                        ps[:, qloA : qloA + P], ps[:, qloA : qloA + P], tri
                    )
                nc.tensor.matmul(
                    ps[:, QB + qloB : 2 * QB],
                    kt_sb[:, jb * P : (jb + 1) * P],
                    qt_sb[:, qs0 + qloB : qs0 + QB],
                    start=True,
                    stop=True,
                )
                drain(2 * pi * win // (2 * npr))
                if db >= 0:
                    nc.vector.tensor_add(
                        ps[:, QB + qloB : QB + qloB + P],
                        ps[:, QB + qloB : QB + qloB + P],
                        tri,
                    )
                expt = expp.tile(
                    [P, 2 * QB], dt_av, name=f"exp{g}_{pi}", tag="expst"
                )
                nc.scalar.activation(
                    expt[:, qloA : 2 * QB],
                    ps[:, qloA : 2 * QB],
                    mybir.ActivationFunctionType.Exp,
                )
                # zero the fully-masked strips so acc/sums see exact zeros
                # (the un-laned tail pair skips this: nothing reads them)
                if lane_seq[pi] is not None:
                    if qloA > 0:
                        nc.gpsimd.memset(expt[:, 0:qloA], 0.0)
                    if qloB > 0:
                        nc.gpsimd.memset(expt[:, QB : QB + qloB], 0.0)
                if pi == 2 and deferred:
                    for op in deferred:
                        op()
                    deferred = []
                drain((2 * pi + 1) * win // (2 * npr))
                nc.tensor.matmul(
                    pav[:, qloA:QB],
                    v_sb[:, ja * H : (ja + 1) * H],
                    expt[:, qloA:QB],
                    start=(pi == 0),
                    stop=False,
                    skip_group_check=True,
                )
                nc.tensor.matmul(
                    pav[:, qloB:QB],
                    v_sb[:, jb * H : (jb + 1) * H],
                    expt[:, QB + qloB : 2 * QB],
                    start=False,
                    stop=(pi == npr - 1),
                    skip_group_check=True,
                )
                lk = lane_seq[pi]
                if lk is not None:
                    acc_tile(lk, expt)
                else:
                    last_exp_tile[0] = (expt, qloA, qloB)
                # last block: stream the sums matmuls early; each lane's acc
                # (and finally the last pair) folds into one accumulating
                # PSUM group, keeping DVE off the kernel tail.
                if g == NQB - 1 and pi == npr - 3:
                    acc_o = lanes["o"]["acc"]  # pairs 0,2 complete
                    pss3 = ps_proj.tile([1, QB], f32, name="pss3", tag="proj")
                    pss3_state[0] = pss3
                    nc.tensor.matmul(
                        pss3, ones_col, acc_o[:, 0:QB],
                        start=True, stop=False, skip_group_check=True,
                    )
                    nc.tensor.matmul(
                        pss3, ones_col, acc_o[:, QB : 2 * QB],
                        start=False, stop=False, skip_group_check=True,
                    )
                if g == NQB - 1 and pi == npr - 2:
                    acc_e = lanes["e"]["acc"]  # pairs 0,2,4,6 complete
                    pss3 = pss3_state[0]
                    nc.tensor.matmul(
                        pss3, ones_col, acc_e[:, 0:QB],
                        start=False, stop=False, skip_group_check=True,
                    )
                    nc.tensor.matmul(
                        pss3, ones_col, acc_e[:, QB : 2 * QB],
                        start=False, stop=False, skip_group_check=True,
                    )
            for op in ops_q:  # leftovers (shouldn't happen)
                op()

            if g == NQB - 1:
                # tail: last pair folds straight into the sums matmul
                # (valid spans only, so the garbage strips never matter)
                lt, lqa, lqb = last_exp_tile[0]
                pss3 = pss3_state[0]
                nc.tensor.matmul(
                    pss3[:, lqa:QB], ones_col, lt[:, lqa:QB],
                    start=False, stop=False, skip_group_check=True,
                )
                nc.tensor.matmul(
                    pss3[:, lqb:QB], ones_col, lt[:, QB + lqb : 2 * QB],
                    start=False, stop=True, skip_group_check=True,
                )
                o3 = outp.tile([P, QB], dt_av, name="o3", tag="o")
                nc.scalar.copy(o3, pav)
                nc.sync.dma_start(pavT[:, qs0 : qs0 + QB], o3)
                nc.scalar.copy(sums_sb[:, qs0 : qs0 + QB], pss3)
                nc.scalar.dma_start(
                    sums[:, qs0 : qs0 + QB], sums_sb[:, qs0 : qs0 + QB]
                )
            else:

                def mk_finalize(g=g, qs0=qs0, lanes=lanes, pav=pav):
                    def fin():
                        le, lo = lanes["e"], lanes["o"]
                        if le["acc"] is None:
                            # block 0: one pair per lane, combine directly
                            acc = accp.tile(
                                [P, 2 * QB], dt_av, name=f"accf{g}", tag="acc"
                            )
                            nc.vector.tensor_add(
                                acc, le["first"][0], lo["first"][0]
                            )
                        else:
                            acc = le["acc"]
                            if lo["acc"] is not None:
                                nc.vector.tensor_add(acc, acc, lo["acc"])
                            elif lo["first"] is not None:
                                nc.vector.tensor_add(acc, acc, lo["first"][0])
                        pss = ps_proj.tile(
                            [1, QB], f32, name=f"pss{g}", tag="proj"
                        )
                        nc.tensor.matmul(
                            pss, ones_col, acc[:, 0:QB],
                            start=True, stop=False, skip_group_check=True,
                        )
                        nc.tensor.matmul(
                            pss, ones_col, acc[:, QB : 2 * QB],
                            start=False, stop=True, skip_group_check=True,
                        )
                        nc.scalar.copy(sums_sb[:, qs0 : qs0 + QB], pss)
                        nc.sync.dma_start(
                            sums[:, qs0 : qs0 + QB], sums_sb[:, qs0 : qs0 + QB]
                        )
                        o = outp.tile([P, QB], dt_av, name=f"o{g}", tag="o")
                        nc.vector.tensor_copy(o, pav)
                        nc.sync.dma_start(pavT[:, qs0 : qs0 + QB], o)

                    return fin

                deferred = [mk_finalize()]
        for op in deferred:
            op()

    nc.compile()
    return nc


def _get_bass():
    if "nc" not in _CACHE:
        _CACHE["nc"] = _build()
    return _CACHE["nc"]


LAST_RESULT = None  # BassKernelResults of the most recent kernel() call


def _make_in_maps(x, Wq, Wk, Wv):
    np_dt = ml_dtypes.bfloat16

    def _wlayout(w):  # [C, H] -> [P, NCC*H]: sbuf layout, contiguous DMA
        w = np.asarray(w, np.float32).reshape(NCC, P, H).transpose(1, 0, 2)
        return np.ascontiguousarray(w.reshape(P, NCC * H)).astype(np_dt)

    wq_s = _wlayout(np.asarray(Wq, np.float32) * SCALE)
    wk_s = _wlayout(Wk)
    wv_s = _wlayout(Wv)
    x = np.asarray(x, np.float32)

    in_maps = []
    for b in range(N_CORES):
        in_maps.append(
            {
                "xT": np.ascontiguousarray(x[b].T).astype(np_dt),
                "wq": wq_s,
                "wk": wk_s,
                "wv": wv_s,
            }
        )
    return in_maps


def _finalize(pavT_arr, sums_arr):
    pav = np.asarray(pavT_arr).astype(np.float32).T  # [T, H]
    s = np.asarray(sums_arr).astype(np.float32).reshape(T, 1)
    return pav / s


def _in_map_for_core(inputs, b):
    return _make_in_maps(**inputs)[b]


def _out_from_core(sim):
    return _finalize(sim.tensor("pavT"), sim.tensor("sums"))


def kernel(x, Wq, Wk, Wv):
    global LAST_RESULT
    from concourse.bass_utils import run_bass_kernel_spmd

    in_maps = _make_in_maps(x, Wq, Wk, Wv)

    nc = _get_bass()
    res = run_bass_kernel_spmd(nc, in_maps, core_ids=list(range(N_CORES)))
    LAST_RESULT = res
    return np.stack(
        [_finalize(r["pavT"], r["sums"]) for r in res.results], axis=0
    )


# revision 33
# speedup vs baseline: 1.0166x; 1.0166x over previous
"""Single-head causal attention (B=8, T=2048, C=1024, H=128) on 8 TRN2 NeuronCores.

Sharding: data-parallel over batch B — core b computes attention for x[b].
Host-side prep per core: x[b] is transposed to xT [C, T] (contraction dim C on
SBUF partitions) and the softmax scale C**-0.5 is folded into Wq. The kernel
returns the UNNORMALIZED attention output pavT [H, T] (bf16) plus the softmax
denominators sums [1, T] (f32); the host divides and untransposes.

Device kernel per core (v2 — pair-tile attention, warmup MMs, staged DMA):
  head: input DMAs staged across the 3 queues (sync/scalar/gpsimd) in
    consumption order; ~16 warmup matmuls on a dummy tile keep the PE HAM
    clock warm through the DMA-bound head so the real stream runs at 2.4 GHz.
  quarter 0 projections c-major over arriving x chunks (V,Q,K per chunk).
  attention per q-block g over PAIRS of s-tiles (j, j+1):
      ST pair = two matmuls into one 2-bank PSUM tile [128, 1024]
      diag boundary subtiles get triangular mask adds (DVE, in PSUM)
      ONE exp ACTIVATE over the pair -> SBUF bf16 (halves ACT overhead);
      masked strips zeroed by DVE memsets
      pav += V_j.T @ exp per tile [H, 512] PSUM accumulate (PE)
      running exp-sums: pair-width tensor_adds, DVE (even pairs) / GpSimd
      interleaved projection matmuls fill the exp latency
  blocks 0-2 finalize deferred into the next block (fold + ones-col matmul +
  copies + DMA). block 3 finalizes sums EARLY via accumulating ones-col
  matmuls into PSUM, so the kernel tail is just 2 matmuls + copy + DMA.
"""

from contextlib import ExitStack

import numpy as np
import ml_dtypes

B, T, C, H = 8, 2048, 1024, 128
P = 128
NT = T // P  # 16 s-tiles
NCC = C // P  # 8 contraction chunks
QB = 512  # q-block width
NQB = T // QB  # 4 q-blocks / projection quarters
N_CORES = 8
SCALE = float(C) ** -0.5

_CACHE = {}


def _build():
    import concourse.bass as bass
    import concourse.mybir as mybir
    import concourse.tile as tile
    from concourse import bacc

    dt = mybir.dt
    dt_in = dt.bfloat16
    dt_av = dt.bfloat16
    f32 = dt.float32

    nc = bacc.Bacc("TRN2", target_bir_lowering=False, debug=False)
    xT = nc.dram_tensor("xT", [C, T], dt_in, kind="ExternalInput").ap()
    wq = nc.dram_tensor("wq", [P, NCC * H], dt_in, kind="ExternalInput").ap()
    wk = nc.dram_tensor("wk", [P, NCC * H], dt_in, kind="ExternalInput").ap()
    wv = nc.dram_tensor("wv", [P, NCC * H], dt_in, kind="ExternalInput").ap()
    pavT = nc.dram_tensor("pavT", [H, T], dt_av, kind="ExternalOutput").ap()
    sums = nc.dram_tensor("sums", [1, T], f32, kind="ExternalOutput").ap()

    with tile.TileContext(nc) as tc, ExitStack() as ctx:
        wpool = ctx.enter_context(tc.tile_pool(name="wpool", bufs=1))
        w_sb = {
            name: wpool.tile([P, NCC * H], dt_in, name=f"{name}_sb")
            for name in ("wv", "wq", "wk")
        }
        xpool = ctx.enter_context(tc.tile_pool(name="xpool", bufs=1))
        xt_sb = xpool.tile([P, NCC * T], dt_in)
        xt_v = xt_sb.rearrange("p (c t) -> p c t", c=NCC)
        xT_v = xT.rearrange("(c p) t -> p c t", c=NCC)

        def x_chunk(eng, c0, c1, t0, t1):
            eng.dma_start(xt_v[:, c0:c1, t0:t1], xT_v[:, c0:c1, t0:t1])

        # --- input DMAs: quarter-0 pieces spread in consumption order over
        # the 3 queues, with per-queue byte counts balanced (~615KB each) so
        # all queues finish the quarter-0 phase together and the bulk n=1..3
        # loads (queued behind) don't steal bandwidth from it ---
        x_chunk(nc.sync, 0, 1, 0, 512)  # c0 first: gates the first proj MM
        nc.scalar.dma_start(w_sb["wv"], wv)
        x_chunk(nc.gpsimd, 1, 2, 0, 512)
        x_chunk(nc.scalar, 2, 3, 0, 512)
        x_chunk(nc.sync, 3, 4, 0, 512)
        nc.gpsimd.dma_start(w_sb["wq"], wq)
        x_chunk(nc.scalar, 5, 6, 0, 512)
        x_chunk(nc.sync, 6, 7, 0, 512)
        x_chunk(nc.gpsimd, 4, 5, 0, 512)
        nc.sync.dma_start(w_sb["wk"], wk)
        x_chunk(nc.scalar, 7, 8, 0, 512)
        # n=1..3: behind the quarter-0 items in each queue's FIFO; n=1 split
        # over all three queues (needed by Q(1)/V(1) proj during block 0)
        x_chunk(nc.gpsimd, 0, 3, 512, 1024)
        x_chunk(nc.scalar, 3, 6, 512, 1024)
        x_chunk(nc.sync, 6, 8, 512, 1024)
        x_chunk(nc.gpsimd, 0, 8, 1024, 1536)
        x_chunk(nc.sync, 0, 8, 1536, 2048)

        consts = ctx.enter_context(tc.tile_pool(name="consts", bufs=1))
        # triangular boundary mask: tri[s, q] = -30000 where q < s else 0
        tri = consts.tile([P, P], f32)
        nc.gpsimd.memset(tri, 0.0)
        nc.gpsimd.affine_select(
            out=tri,
            in_=tri,
            compare_op=mybir.AluOpType.is_ge,
            fill=-30000.0,
            base=0,
            pattern=[[1, P]],
            channel_multiplier=-1,
        )
        ones_col = consts.tile([P, 1], dt_av)
        nc.vector.memset(ones_col, 1.0)
        warm = consts.tile([P, QB], dt_in)
        nc.vector.memset(warm, 0.002)

        qkv = ctx.enter_context(tc.tile_pool(name="qkv", bufs=1))
        qt_sb = qkv.tile([P, T], dt_in)
        kt_sb = qkv.tile([P, T], dt_in)
        vt_sb = qkv.tile([P, T], dt_av)
        vpool = ctx.enter_context(tc.tile_pool(name="vpool", bufs=1))
        v_sb = vpool.tile([P, NT * H], dt_av)

        # PSUM budget (8 banks): score pairs 2x2 (also hosts q0/k0 proj),
        # proj 2x1 (also hosts warmup + sums tiles), pav 2x1
        ps_sc = ctx.enter_context(tc.tile_pool(name="ps_sc", bufs=2, space="PSUM"))
        ps_proj = ctx.enter_context(tc.tile_pool(name="ps_proj", bufs=2, space="PSUM"))
        ps_av = ctx.enter_context(tc.tile_pool(name="ps_av", bufs=2, space="PSUM"))

        expp = ctx.enter_context(tc.tile_pool(name="expp", bufs=10))
        outp = ctx.enter_context(tc.tile_pool(name="outp", bufs=2))
        accp = ctx.enter_context(tc.tile_pool(name="accp", bufs=4))
        sums_sb_pool = ctx.enter_context(tc.tile_pool(name="sums_sb", bufs=1))
        sums_sb = sums_sb_pool.tile([1, T], f32)

        # warmup: ONE accumulation group into one proj-pool bank, so the MMs
        # run back-to-back (no WAW semaphores) and HAM reaches 8/8 early.
        # All upfront: ~10 x 427ns cold > the 3.4us HAM SHORT window.
        N_WARM = 24
        warm_ps = ps_proj.tile([P, QB], f32, name="warm_ps", tag="proj")
        for i in range(N_WARM):
            nc.tensor.matmul(
                warm_ps, warm[:, 0:P], warm,
                start=(i == 0), stop=(i == N_WARM - 1),
                skip_group_check=True,
            )

        def proj_ops(pname, dst, n, copy_eng, do_transpose, pool=None, ptag=None):
            """Closures: 8 proj matmuls + copy (+ quarter transpose)."""
            wt = w_sb[pname]
            state = {}
            pool = pool if pool is not None else ps_proj
            ptag = ptag if ptag is not None else "proj"

            def mk_mm(c):
                def op():
                    if c == 0:
                        state["ps"] = pool.tile(
                            [P, QB], f32, name=f"ps_{pname}{n}", tag=ptag
                        )
                    nc.tensor.matmul(
                        state["ps"],
                        wt[:, c * H : (c + 1) * H],
                        xt_sb[:, c * T + n * QB : c * T + (n + 1) * QB],
                        start=(c == 0),
                        stop=(c == NCC - 1),
                    )

                return op

            def cp():
                fn = (
                    copy_eng.tensor_copy if copy_eng is nc.vector else copy_eng.copy
                )
                fn(dst[:, n * QB : (n + 1) * QB], state["ps"])

            ops = [mk_mm(c) for c in range(NCC)] + [cp]
            if do_transpose:

                def tr():
                    nc.sync.dma_start(
                        v_sb[:, 4 * n * H : 4 * (n + 1) * H].rearrange(
                            "p (t h) -> p t h", t=4
                        ),
                        vt_sb[:, n * QB : (n + 1) * QB],
                        transpose=True,
                    )

                ops.append(tr)
            return ops

        # --- quarter 0 up front, GROUP-major (V, then Q, then K): the V
        # path has the longest tail (copy + DMA-transpose + receipt), so V's
        # group must finish first; each group's copy is emitted immediately.
        # Q0/K0 accumulate in the (idle until block 0) score-pair banks so
        # the three groups don't fight over the two proj banks.
        v0 = proj_ops("wv", vt_sb, 0, nc.scalar, True)
        q0 = proj_ops("wq", qt_sb, 0, nc.vector, False, pool=ps_sc, ptag="sc")
        k0 = proj_ops("wk", kt_sb, 0, nc.vector, False, pool=ps_sc, ptag="sc")
        for op in v0 + q0 + k0:
            op()

        # --- attention blocks (pair-tile) ---
        deferred = []  # sums finalization + outputs of the previous block
        for g in range(NQB):
            qs0 = g * QB
            njt = 4 * g + 4
            npr = njt // 2
            pav = ps_av.tile([P, QB], f32, name=f"pav{g}", tag="ps_av")

            # pair processing order: clean pairs ascending, diag pairs last;
            # last block puts the diag pairs mid-block so the kernel ends on
            # dense full tiles
            pairs = [(2 * i, 2 * i + 1) for i in range(npr)]
            if g == NQB - 1:
                # diag pairs mid-block: the kernel ends on dense full pairs
                # whose PE work overlaps the diag mask->exp chains
                pairs = pairs[0:4] + pairs[6:8] + pairs[4:6]
                win = 4  # K(3)/V(3) must land before pair idx 4 (j=12)
            else:
                win = max(npr - 1, 1)

            # deadline-ordered proj op queue for this block.  V(3) is
            # projected in block 3 itself (not block 2): block 2 is
            # PE-bound and block 3 ACT-bound, so this balances them.
            # Its copy goes on DVE to keep ACT free for block-3 exps.
            ops_q = []
            if g == NQB - 1:
                # V(3) first: its copy+transpose+receipt chain is longest
                ops_q += proj_ops("wv", vt_sb, g, nc.vector, True)
            if g >= 1:
                ops_q += proj_ops("wk", kt_sb, g, nc.vector, False)
            if g + 1 < NQB:
                ops_q += proj_ops("wq", qt_sb, g + 1, nc.vector, False)
                if g + 1 < NQB - 1:
                    ops_q += proj_ops("wv", vt_sb, g + 1, nc.scalar, True)

            # acc lanes: even pairs -> DVE, odd pairs -> GpSimd.  For the
            # last block, pair npr-1 is left out of the lanes (it feeds the
            # ones-matmul tail directly).
            lanes = {
                "e": {"eng": nc.vector, "first": None, "acc": None},
                "o": {"eng": nc.gpsimd, "first": None, "acc": None},
            }
            # GpSimd ("o") takes EARLY pairs only — its adds are ~3x slower
            # than DVE, so each block's final adds (which feed the sums
            # matmuls) always land on DVE ("e").
            lane_seq = []
            for pi in range(npr):
                if g == NQB - 1 and pi == npr - 1:
                    lane_seq.append(None)
                elif g == NQB - 1:
                    lane_seq.append("o" if pi in (0, 2) else "e")
                else:
                    lane_seq.append("o" if pi % 2 == 0 else "e")

            last_exp_tile = [None]
            pss3_state = [None]

            def acc_tile(lane_key, expt):
                ln = lanes[lane_key]
                eng = ln["eng"]
                if ln["acc"] is None and ln["first"] is None:
                    ln["first"] = (expt,)
                elif ln["acc"] is None:
                    acc = accp.tile(
                        [P, 2 * QB], dt_av, name=f"acc{g}{lane_key}", tag="acc"
                    )
                    # init: acc = first + this (full pair width; strips zeroed)
                    eng.tensor_add(acc, ln["first"][0], expt)
                    ln["acc"] = acc
                    ln["first"] = None
                else:
                    eng.tensor_add(ln["acc"], ln["acc"], expt)

            def drain(k):
                if ops_q and k < win:
                    take = -(-len(ops_q) // (win - k))
                    for op in ops_q[:take]:
                        op()
                    del ops_q[:take]

            for pi, (ja, jb) in enumerate(pairs):
                da, db = ja - 4 * g, jb - 4 * g
                qloA = max(0, P * da)
                qloB = max(0, P * db)
                ps = ps_sc.tile([P, 2 * QB], f32, name=f"ps_{g}_{pi}", tag="sc")
                nc.tensor.matmul(
                    ps[:, qloA:QB],
                    kt_sb[:, ja * P : (ja + 1) * P],
                    qt_sb[:, qs0 + qloA : qs0 + QB],
                    start=True,
                    stop=True,
                )
                if da >= 0:
                    nc.vector.tensor_add(
                        ps[:, qloA : qloA + P], ps[:, qloA : qloA + P], tri
                    )
                nc.tensor.matmul(
                    ps[:, QB + qloB : 2 * QB],
                    kt_sb[:, jb * P : (jb + 1) * P],
                    qt_sb[:, qs0 + qloB : qs0 + QB],
                    start=True,
                    stop=True,
                )
                drain(2 * pi * win // (2 * npr))
                if db >= 0:
                    nc.vector.tensor_add(
                        ps[:, QB + qloB : QB + qloB + P],
                        ps[:, QB + qloB : QB + qloB + P],
                        tri,
                    )
                expt = expp.tile(
                    [P, 2 * QB], dt_av, name=f"exp{g}_{pi}", tag="expst"
                )
                nc.scalar.activation(
                    expt[:, qloA : 2 * QB],
                    ps[:, qloA : 2 * QB],
                    mybir.ActivationFunctionType.Exp,
                )
                # zero the fully-masked strips so acc/sums see exact zeros
                # (the un-laned tail pair skips this: nothing reads them)
                if lane_seq[pi] is not None:
                    if qloA > 0:
                        nc.gpsimd.memset(expt[:, 0:qloA], 0.0)
                    if qloB > 0:
                        nc.gpsimd.memset(expt[:, QB : QB + qloB], 0.0)
                if pi == 2 and deferred:
                    for op in deferred:
                        op()
                    deferred = []
                drain((2 * pi + 1) * win // (2 * npr))
                nc.tensor.matmul(
                    pav[:, qloA:QB],
                    v_sb[:, ja * H : (ja + 1) * H],
                    expt[:, qloA:QB],
                    start=(pi == 0),
                    stop=False,
                    skip_group_check=True,
                )
                nc.tensor.matmul(
                    pav[:, qloB:QB],
                    v_sb[:, jb * H : (jb + 1) * H],
                    expt[:, QB + qloB : 2 * QB],
                    start=False,
                    stop=(pi == npr - 1),
                    skip_group_check=True,
                )
                lk = lane_seq[pi]
                if lk is not None:
                    acc_tile(lk, expt)
                else:
                    last_exp_tile[0] = (expt, qloA, qloB)
                # last block: stream the sums matmuls early; each lane's acc
                # (and finally the last pair) folds into one accumulating
                # PSUM group, keeping DVE off the kernel tail.
                if g == NQB - 1 and pi == npr - 3:
                    acc_o = lanes["o"]["acc"]  # pairs 0,2 complete
                    pss3 = ps_proj.tile([1, QB], f32, name="pss3", tag="proj")
                    pss3_state[0] = pss3
                    nc.tensor.matmul(
                        pss3, ones_col, acc_o[:, 0:QB],
                        start=True, stop=False, skip_group_check=True,
                    )
                    nc.tensor.matmul(
                        pss3, ones_col, acc_o[:, QB : 2 * QB],
                        start=False, stop=False, skip_group_check=True,
                    )
                if g == NQB - 1 and pi == npr - 2:
                    acc_e = lanes["e"]["acc"]  # pairs 0,2,4,6 complete
                    pss3 = pss3_state[0]
                    nc.tensor.matmul(
                        pss3, ones_col, acc_e[:, 0:QB],
                        start=False, stop=False, skip_group_check=True,
                    )
                    nc.tensor.matmul(
                        pss3, ones_col, acc_e[:, QB : 2 * QB],
                        start=False, stop=False, skip_group_check=True,
                    )
            for op in ops_q:  # leftovers (shouldn't happen)
                op()

            if g == NQB - 1:
                # tail: last pair folds straight into the sums matmul
                # (valid spans only, so the garbage strips never matter)
                lt, lqa, lqb = last_exp_tile[0]
                pss3 = pss3_state[0]
                nc.tensor.matmul(
                    pss3[:, lqa:QB], ones_col, lt[:, lqa:QB],
                    start=False, stop=False, skip_group_check=True,
                )
                nc.tensor.matmul(
                    pss3[:, lqb:QB], ones_col, lt[:, QB + lqb : 2 * QB],
                    start=False, stop=True, skip_group_check=True,
                )
                o3 = outp.tile([P, QB], dt_av, name="o3", tag="o")
                nc.scalar.copy(o3, pav)
                nc.sync.dma_start(pavT[:, qs0 : qs0 + QB], o3)
                nc.scalar.copy(sums_sb[:, qs0 : qs0 + QB], pss3)
                nc.scalar.dma_start(
                    sums[:, qs0 : qs0 + QB], sums_sb[:, qs0 : qs0 + QB]
                )
            else:

                def mk_finalize(g=g, qs0=qs0, lanes=lanes, pav=pav):
                    def fin():
                        le, lo = lanes["e"], lanes["o"]
                        if le["acc"] is None:
                            # block 0: one pair per lane, combine directly
                            acc = accp.tile(
                                [P, 2 * QB], dt_av, name=f"accf{g}", tag="acc"
                            )
                            nc.vector.tensor_add(
                                acc, le["first"][0], lo["first"][0]
                            )
                        else:
                            acc = le["acc"]
                            if lo["acc"] is not None:
                                nc.vector.tensor_add(acc, acc, lo["acc"])
                            elif lo["first"] is not None:
                                nc.vector.tensor_add(acc, acc, lo["first"][0])
                        pss = ps_proj.tile(
                            [1, QB], f32, name=f"pss{g}", tag="proj"
                        )
                        nc.tensor.matmul(
                            pss, ones_col, acc[:, 0:QB],
                            start=True, stop=False, skip_group_check=True,
                        )
                        nc.tensor.matmul(
                            pss, ones_col, acc[:, QB : 2 * QB],
                            start=False, stop=True, skip_group_check=True,
                        )
                        nc.scalar.copy(sums_sb[:, qs0 : qs0 + QB], pss)
                        nc.sync.dma_start(
                            sums[:, qs0 : qs0 + QB], sums_sb[:, qs0 : qs0 + QB]
                        )
                        o = outp.tile([P, QB], dt_av, name=f"o{g}", tag="o")
                        nc.vector.tensor_copy(o, pav)
                        nc.sync.dma_start(pavT[:, qs0 : qs0 + QB], o)

                    return fin

                deferred = [mk_finalize()]
        for op in deferred:
            op()

    nc.compile()
    return nc


def _get_bass():
    if "nc" not in _CACHE:
        _CACHE["nc"] = _build()
    return _CACHE["nc"]


LAST_RESULT = None  # BassKernelResults of the most recent kernel() call


def _make_in_maps(x, Wq, Wk, Wv):
    np_dt = ml_dtypes.bfloat16

    def _wlayout(w):  # [C, H] -> [P, NCC*H]: sbuf layout, contiguous DMA
        w = np.asarray(w, np.float32).reshape(NCC, P, H).transpose(1, 0, 2)
        return np.ascontiguousarray(w.reshape(P, NCC * H)).astype(np_dt)

    wq_s = _wlayout(np.asarray(Wq, np.float32) * SCALE)
    wk_s = _wlayout(Wk)
    wv_s = _wlayout(Wv)
    x = np.asarray(x, np.float32)

    in_maps = []
    for b in range(N_CORES):
        in_maps.append(
            {
                "xT": np.ascontiguousarray(x[b].T).astype(np_dt),
                "wq": wq_s,
                "wk": wk_s,
                "wv": wv_s,
            }
        )
    return in_maps


def _finalize(pavT_arr, sums_arr):
    pav = np.asarray(pavT_arr).astype(np.float32).T  # [T, H]
    s = np.asarray(sums_arr).astype(np.float32).reshape(T, 1)
    return pav / s


def _in_map_for_core(inputs, b):
    return _make_in_maps(**inputs)[b]


def _out_from_core(sim):
    return _finalize(sim.tensor("pavT"), sim.tensor("sums"))


def kernel(x, Wq, Wk, Wv):
    global LAST_RESULT
    from concourse.bass_utils import run_bass_kernel_spmd

    in_maps = _make_in_maps(x, Wq, Wk, Wv)

    nc = _get_bass()
    res = run_bass_kernel_spmd(nc, in_maps, core_ids=list(range(N_CORES)))
    LAST_RESULT = res
    return np.stack(
        [_finalize(r["pavT"], r["sums"]) for r in res.results], axis=0
    )
